# revision 7
# baseline (speedup 1.0000x reference)
"""Causal self-attention on 8 TRN2 NeuronCores — no-collective version.

Sharding: batch x head-group mesh (4 x 2). Core c handles batch b = c//2
and head group g = c%2 (8 of 16 heads). Each core computes its partial
projection out_partial = y(8 heads) @ W_proj[rows(g)] over the FULL
sequence; the host sums the two partials of each pair while unsharding.
No on-device collective -> cores are fully independent (no cross-core
rendezvous, no bounce DRAM traffic).

Host-side data marshalling (part of sharding):
  xT   [128, 8, 2048] bf16 : x[b].T packed partition-major (d%128 -> p)
  wqk  [128, 8, 8, 128] bf16: q/k weight col-tiles, stationary-ready
  wav  [128, 8, 512] bf16  : v weight, stationary-contraction packed
  wp   [128, 4, 1024] bf16 : W_proj rows(g) packed
  out  [128, 16, 1024] f32 : partial projection, partition-major rows

Device per core:
  qkv = bf16 matmuls from xT (no on-device transpose)
  per head: S^T = k^T.T @ q^T, P = exp(S/8) causal, y = P@v with a
  ones-column giving the softmax denominator l for free; 1/l broadcast
  across partitions via a ones-stationary PE matmul (no DRAM bounce);
  partial_out = y @ wp staged in SBUF, written in 4 contiguous DMAs.
"""

from contextlib import ExitStack

import numpy as np
import ml_dtypes

import concourse.bass as bass
import concourse.tile as tile
from concourse import bacc, mybir
from concourse.bass_utils import run_bass_kernel_spmd

F32 = mybir.dt.float32
F32R = mybir.dt.float32r
BF16 = mybir.dt.bfloat16
AF = mybir.ActivationFunctionType
BF16_NP = np.dtype(ml_dtypes.bfloat16)

D = 1024          # model dim
T = 2048          # sequence length
B = 4             # batch
HD = 64           # head dim
NH = 8            # heads per core
DC = D // 128     # 8 contraction chunks
TT = T // 128     # 16 t-tiles
SCALE = 1.0 / 8.0  # 1/sqrt(HD)


def _pieces(width):
    """Split width into matmul pieces <=512, PSUM-bank aligned from offset 0."""
    out = []
    off = 0
    while off < width:
        n = min(512, width - off)
        out.append((off, n))
        off += n
    return out


def build(repeat=1, collective=False):
    nc = bacc.Bacc("TRN2", target_bir_lowering=False, debug=False, num_devices=1)

    xT_ext = nc.dram_tensor("xT", [128, DC, T], BF16, kind="ExternalInput").ap()
    wqk_ext = nc.dram_tensor("wqk", [128, 8, DC, 128], BF16, kind="ExternalInput").ap()
    wav_ext = nc.dram_tensor("wav", [128, DC, 512], BF16, kind="ExternalInput").ap()
    wp_ext = nc.dram_tensor("wp", [128, 4, D], BF16, kind="ExternalInput").ap()
    mask_ext = nc.dram_tensor("trimask", [128, 128], BF16, kind="ExternalInput").ap()
    out_ext = nc.dram_tensor("out_shard", [128, TT, D], F32, kind="ExternalOutput").ap()

    with tile.TileContext(nc) as tc, ExitStack() as top:
        # ---- persistent tiles ----
        pers = top.enter_context(tc.tile_pool(name="pers", bufs=1))

        qkT = [pers.tile([128, T], BF16, tag=f"qkT{i}", name=f"qkT{i}") for i in range(8)]
        # v_sb[tt]: [128 k-parts, 8 heads, 64 v + 1 ones] bf16
        v_sb = [pers.tile([128, NH, HD + 1], BF16, tag=f"v{i}", name=f"v{i}") for i in range(TT)]
        mask_bf = pers.tile([128, 128], BF16, tag="maskbf")
        ones_st = pers.tile([128, 64], F32, tag="ones_st")
        ones64 = pers.tile([128, 64], F32R, tag="ones64")
        wp_sb = pers.tile([128, 4, D], BF16, tag="wp")

        nc.sync.dma_start(mask_bf[:], mask_ext)
        nc.sync.dma_start(wp_sb[:], wp_ext)
        nc.vector.memset(ones_st[:], 1.0)
        nc.vector.tensor_copy(ones64[:], ones_st[:])

        def body(iv=None):
            # ================= phase 1: QKV =================
            with ExitStack() as ph1:
                p1 = ph1.enter_context(tc.tile_pool(name="p1", bufs=1))
                qkvps = ph1.enter_context(
                    tc.tile_pool(name="qkvps", bufs=3, space="PSUM"))

                xT = p1.tile([128, DC, T], BF16, tag="xT", name="xT")
                wqk = p1.tile([128, 8, DC, 128], BF16, tag="wqk", name="wqk")
                wav = p1.tile([128, DC, 512], BF16, tag="wav", name="wav")
                nc.sync.dma_start(wqk[:], wqk_ext)
                nc.sync.dma_start(xT[:], xT_ext)
                nc.sync.dma_start(wav[:], wav_ext)

                # --- q/k col-tiles: qkT[ct] = wa[:, ct].T @ x.T ---
                # emit in order q0,k0,q1,k1,... so early heads finish first
                for ct in [0, 4, 1, 5, 2, 6, 3, 7]:
                    for tch in range(4):
                        ps = qkvps.tile([128, 512], F32, tag="qkvps")
                        for dc in range(DC):
                            nc.tensor.matmul(
                                ps[:],
                                wqk[:, ct, dc, :],
                                xT[:, dc, tch * 512:(tch + 1) * 512],
                                start=(dc == 0), stop=(dc == DC - 1))
                        nc.any.tensor_copy(
                            qkT[ct][:, tch * 512:(tch + 1) * 512], ps[:])

                # --- v natural: v[t-tile] = x[t-tile] @ wa_v ---
                for tt in range(TT):
                    ps = qkvps.tile([128, 512], F32, tag="qkvps")
                    for dc in range(DC):
                        nc.tensor.matmul(
                            ps[:],
                            xT[:, dc, tt * 128:(tt + 1) * 128],
                            wav[:, dc, :],
                            start=(dc == 0), stop=(dc == DC - 1))
                    nc.any.tensor_copy(
                        v_sb[tt][:, :, 0:HD],
                        ps[:].rearrange("p (h d) -> p h d", h=NH))
                    nc.vector.memset(v_sb[tt][:, :, HD:HD + 1], 1.0)

            # ================= phase 2: attention =================
            with ExitStack() as ph23:
                p23 = ph23.enter_context(tc.tile_pool(name="p23", bufs=1))
                yT = [p23.tile([128, T], BF16, tag=f"yT{i}", name=f"yT{i}")
                      for i in range(4)]
                ph2 = ph23.enter_context(ExitStack())
                sps = ph2.enter_context(
                    tc.tile_pool(name="sps", bufs=2, space="PSUM"))
                yps = ph2.enter_context(
                    tc.tile_pool(name="yps", bufs=2, space="PSUM"))
                ppool = ph2.enter_context(tc.tile_pool(name="ppool", bufs=3))
                npool = ph2.enter_context(tc.tile_pool(name="npool", bufs=2))

                for h in range(NH):
                    ct_q, ct_k = h // 2, 4 + h // 2
                    p0 = 64 * (h % 2)
                    qT_h = qkT[ct_q][p0:p0 + 64, :]
                    kT_h = qkT[ct_k][p0:p0 + 64, :]
                    for qh in range(2):
                        qbase = 1024 * qh
                        y_ps = yps.tile([65, 1024], F32, tag="y")
                        n_k = 8 * qh + 8
                        for i in range(n_k):
                            q0 = max(qbase, 128 * i)
                            w = qbase + 1024 - q0
                            s_ps = sps.tile([128, w], F32, tag="s")
                            for off, n in _pieces(w):
                                nc.tensor.matmul(
                                    s_ps[:, off:off + n],
                                    kT_h[:, 128 * i:128 * (i + 1)],
                                    qT_h[:, q0 + off:q0 + off + n],
                                    start=True, stop=True)
                            p_sb = ppool.tile([128, 1024], BF16, tag="p")
                            nc.scalar.activation(
                                p_sb[:, 0:w], s_ps[:], AF.Exp, scale=SCALE)
                            if 128 * i >= qbase:  # diagonal block: causal mask
                                nc.vector.tensor_mul(
                                    p_sb[:, 0:128], p_sb[:, 0:128], mask_bf[:])
                            # AV pieces: align to y_ps's 512-wide PSUM banks
                            yoff = q0 - qbase
                            aoff = 0
                            while aoff < w:
                                n = min(512 - (yoff + aoff) % 512, w - aoff)
                                bank = (yoff + aoff) // 512
                                nc.tensor.matmul(
                                    y_ps[:, yoff + aoff:yoff + aoff + n],
                                    v_sb[i][:, h, :],
                                    p_sb[:, aoff:aoff + n],
                                    start=(i == 0),
                                    stop=(i == 8 * qh + 4 * bank + 3))
                                aoff += n
                        # normalize: yT[h//2][64*(h%2):, qbase:+1024] = y/l
                        # l lives on lane 64; broadcast to lanes 0-63 via a
                        # ones-stationary PE matmul (1-partition contraction),
                        # then reciprocal on all 64 lanes at once.
                        l_r = npool.tile([65, 1024], F32R, tag="lr")
                        nc.vector.tensor_copy(l_r[64:65, :], y_ps[64:65, :])
                        rb_ps = sps.tile([64, 1024], F32, tag="s")
                        for off in (0, 512):
                            nc.tensor.matmul(
                                rb_ps[:, off:off + 512],
                                ones64[64:65, :],
                                l_r[64:65, off:off + 512],
                                start=True, stop=True)
                        recip_b = npool.tile([64, 1024], F32, tag="recipb")
                        nc.vector.reciprocal(recip_b[:], rb_ps[:])
                        if h % 2 == 0:
                            nc.vector.tensor_mul(
                                yT[h // 2][0:64, qbase:qbase + 1024],
                                y_ps[0:64, :], recip_b[:])
                        else:
                            ytmp = npool.tile([64, 1024], BF16, tag="ytmp")
                            nc.vector.tensor_mul(
                                ytmp[:], y_ps[0:64, :], recip_b[:])
                            nc.sync.dma_start(
                                yT[h // 2][64:128, qbase:qbase + 1024],
                                ytmp[:])

                ph2.close()

                # ================= phase 3: proj =================
                ph3 = ph23.enter_context(ExitStack())
                ops = ph3.enter_context(
                    tc.tile_pool(name="ops", bufs=6, space="PSUM"))
                stpool = ph3.enter_context(tc.tile_pool(name="stpool", bufs=2))

                for grp in range(4):
                    stage = stpool.tile([128, 4, D], F32, tag="stg")
                    for j in range(4):
                        tt = grp * 4 + j
                        o_ps = ops.tile([128, 512], F32, tag="o")
                        o_ps2 = ops.tile([128, 512], F32, tag="o")
                        for oc, ps in enumerate([o_ps, o_ps2]):
                            for dc in range(4):
                                nc.tensor.matmul(
                                    ps[:],
                                    yT[dc][:, tt * 128:(tt + 1) * 128],
                                    wp_sb[:, dc, oc * 512:(oc + 1) * 512],
                                    start=(dc == 0), stop=(dc == 3))
                        nc.any.tensor_copy(stage[:, j, 0:512], o_ps[:])
                        nc.any.tensor_copy(stage[:, j, 512:1024], o_ps2[:])
                    nc.sync.dma_start(
                        out_ext[:, grp * 4:(grp + 1) * 4, :], stage[:])

        if repeat == 1:
            body()
        else:
            with tc.For_i(0, repeat, 1) as iv:
                body(iv)

    nc.compile()
    return nc


def make_in_maps(x, W_attn, W_proj):
    trimask = np.triu(np.ones((128, 128), dtype=np.float32)).astype(BF16_NP)
    # xT packed per batch: [128, 8, 2048], xTp[p, dc, t] = x[b, t, dc*128+p]
    xTs = []
    for b in range(B):
        xT = np.ascontiguousarray(
            x[b].T.reshape(DC, 128, T).transpose(1, 0, 2)).astype(BF16_NP)
        xTs.append(xT)
    in_maps = []
    for c in range(8):
        b, g = c // 2, c % 2
        # q cols then k cols for this head group, as 8 col-tiles of 128
        qk_cols = np.concatenate(
            [W_attn[:, 512 * g:512 * g + 512],
             W_attn[:, 1024 + 512 * g:1024 + 512 * g + 512]], axis=1)
        # wqk[p, ct, dc, n] = qk_cols[dc*128+p, ct*128+n]
        wqk = np.ascontiguousarray(
            qk_cols.reshape(DC, 128, 8, 128).transpose(1, 2, 0, 3)
        ).astype(BF16_NP)
        v_cols = W_attn[:, 2048 + 512 * g:2048 + 512 * g + 512]
        wav = np.ascontiguousarray(
            v_cols.reshape(DC, 128, 512).transpose(1, 0, 2)).astype(BF16_NP)
        wp = np.ascontiguousarray(
            W_proj[512 * g:512 * (g + 1), :].reshape(4, 128, D).transpose(1, 0, 2)
        ).astype(BF16_NP)
        in_maps.append({
            "xT": xTs[b],
            "wqk": wqk,
            "wav": wav,
            "wp": wp,
            "trimask": trimask,
        })
    return in_maps


_NC_CACHE = {}


def kernel(x, W_attn, W_proj):
    x = np.asarray(x, dtype=np.float32)
    W_attn = np.asarray(W_attn, dtype=np.float32)
    W_proj = np.asarray(W_proj, dtype=np.float32)
    if "nc" not in _NC_CACHE:
        _NC_CACHE["nc"] = build()
    nc = _NC_CACHE["nc"]
    in_maps = make_in_maps(x, W_attn, W_proj)
    res = run_bass_kernel_spmd(nc, in_maps, list(range(8)))
    out = np.empty((B, T, D), dtype=np.float32)
    for b in range(B):
        # out_shard[p, tt, n] = partial_out[tt*128+p, n]; pair-sum = unshard
        acc = res.results[2 * b]["out_shard"] + res.results[2 * b + 1]["out_shard"]
        out[b] = acc.transpose(1, 0, 2).reshape(T, D)
    return out


# revision 9
# speedup vs baseline: 1.5207x; 1.5207x over previous
"""Causal self-attention on 8 TRN2 NeuronCores — no-collective, f32r attn.

Sharding: batch x head-group mesh (4 x 2). Core c handles batch b = c//2
and head group g = c%2 (8 of 16 heads). Each core computes its partial
projection out_partial = y(8 heads) @ W_proj[rows(g)] over the FULL
sequence; the host sums the two partials of each pair while unsharding.
No on-device collective -> cores are fully independent.

Dtypes: host inputs are bf16 (halves DMA). QKV matmuls run bf16
(Ldweights per matmul, hidden under phase-1 copies). Everything after
the QKV PSUM is float32r: q/k/v land in F32R tiles (the PSUM->SBUF copy
does the f32r encode at no extra cost), exp writes F32R, so every
attention and projection matmul is a self-loading f32r matmul — no
standalone Ldweights and full-rate moving streams.

Host-side packing (part of sharding):
  xT   [128, 8, 2048] bf16 : x[b].T packed partition-major
  wqk  [128, 8, 8, 128] bf16: q/k weight col-tiles, stationary-ready
  wav  [128, 8, 512] bf16  : v weights, contraction packed
  wp   [128, 4, 1024] bf16 : W_proj rows(g) packed
  out  [128, 16, 1024] f32 : partial projection, partition-major rows

Per head: S^T = k^T.T @ q^T, P = exp(S/8) causal, y = P@v with a ones
column giving the softmax denominator l for free; l broadcast across
partitions via a ones-stationary PE matmul, reciprocal on 64 lanes.
"""

from contextlib import ExitStack

import numpy as np
import ml_dtypes

import concourse.bass as bass
import concourse.tile as tile
from concourse import bacc, mybir
from concourse.bass_utils import run_bass_kernel_spmd

F32 = mybir.dt.float32
F32R = mybir.dt.float32r
BF16 = mybir.dt.bfloat16
AF = mybir.ActivationFunctionType
BF16_NP = np.dtype(ml_dtypes.bfloat16)

D = 1024          # model dim
T = 2048          # sequence length
B = 4             # batch
HD = 64           # head dim
NH = 8            # heads per core
DC = D // 128     # 8 contraction chunks
TT = T // 128     # 16 t-tiles
SCALE = 1.0 / 8.0  # 1/sqrt(HD)


def _pieces(width):
    """Split width into matmul pieces <=512, PSUM-bank aligned from offset 0."""
    out = []
    off = 0
    while off < width:
        n = min(512, width - off)
        out.append((off, n))
        off += n
    return out


def build(repeat=1, collective=False):
    nc = bacc.Bacc("TRN2", target_bir_lowering=False, debug=False, num_devices=1)

    xT_ext = nc.dram_tensor("xT", [128, DC, T], BF16, kind="ExternalInput").ap()
    wqk_ext = nc.dram_tensor("wqk", [128, 8, DC, 128], BF16, kind="ExternalInput").ap()
    wav_ext = nc.dram_tensor("wav", [128, DC, 512], BF16, kind="ExternalInput").ap()
    wp_ext = nc.dram_tensor("wp", [128, 4, D], BF16, kind="ExternalInput").ap()
    mask_ext = nc.dram_tensor("trimask", [128, 128], BF16, kind="ExternalInput").ap()
    out_ext = nc.dram_tensor("out_shard", [128, TT, D], F32, kind="ExternalOutput").ap()

    with tile.TileContext(nc) as tc, ExitStack() as top:
        # ---- persistent tiles ----
        pers = top.enter_context(tc.tile_pool(name="pers", bufs=1))

        mask_bf = pers.tile([128, 128], BF16, tag="maskbf")
        mask_r = pers.tile([128, 128], F32R, tag="maskr")
        ones_st = pers.tile([128, 64], F32, tag="ones_st")
        ones64 = pers.tile([128, 64], F32R, tag="ones64")

        nc.sync.dma_start(mask_bf[:], mask_ext)
        nc.vector.tensor_copy(mask_r[:], mask_bf[:])
        nc.vector.memset(ones_st[:], 1.0)
        nc.vector.tensor_copy(ones64[:], ones_st[:])

        def body(iv=None):
            with ExitStack() as ph23:
                p23 = ph23.enter_context(tc.tile_pool(name="p23", bufs=1))
                yT = [p23.tile([128, T], F32R, tag=f"yT{i}", name=f"yT{i}")
                      for i in range(4)]

                ph12 = ph23.enter_context(ExitStack())
                p12 = ph12.enter_context(tc.tile_pool(name="p12", bufs=1))
                qkT = [p12.tile([128, T], F32R, tag=f"qkT{i}", name=f"qkT{i}")
                       for i in range(8)]
                # v_sb[tt]: [128 k-parts, 8 heads, 64 v + 1 ones] f32r
                v_sb = [p12.tile([128, NH, HD + 1], F32R, tag=f"v{i}", name=f"v{i}")
                        for i in range(TT)]

                # ================= phase 1: QKV =================
                with ExitStack() as ph1:
                    p1 = ph1.enter_context(tc.tile_pool(name="p1", bufs=1))
                    qkvps = ph1.enter_context(
                        tc.tile_pool(name="qkvps", bufs=3, space="PSUM"))

                    xT = p1.tile([128, DC, T], BF16, tag="xT", name="xT")
                    wqk = p1.tile([128, 8, DC, 128], BF16, tag="wqk", name="wqk")
                    wav = p1.tile([128, DC, 512], BF16, tag="wav", name="wav")
                    nc.sync.dma_start(wqk[:], wqk_ext)
                    nc.sync.dma_start(xT[:], xT_ext)
                    nc.sync.dma_start(wav[:], wav_ext)

                    # --- q/k col-tiles: qkT[ct] = wa[:, ct].T @ x.T ---
                    # emit q0,k0 first so early heads finish first
                    for ct in [0, 4, 1, 5, 2, 6, 3, 7]:
                        for tch in range(4):
                            ps = qkvps.tile([128, 512], F32, tag="qkvps")
                            for dc in range(DC):
                                nc.tensor.matmul(
                                    ps[:],
                                    wqk[:, ct, dc, :],
                                    xT[:, dc, tch * 512:(tch + 1) * 512],
                                    start=(dc == 0), stop=(dc == DC - 1))
                            nc.vector.tensor_copy(
                                qkT[ct][:, tch * 512:(tch + 1) * 512], ps[:])

                    # --- v natural: v[t-tile] = x[t-tile] @ wa_v ---
                    for tt in range(TT):
                        ps = qkvps.tile([128, 512], F32, tag="qkvps")
                        for dc in range(DC):
                            nc.tensor.matmul(
                                ps[:],
                                xT[:, dc, tt * 128:(tt + 1) * 128],
                                wav[:, dc, :],
                                start=(dc == 0), stop=(dc == DC - 1))
                        nc.vector.tensor_copy(
                            v_sb[tt][:, :, 0:HD],
                            ps[:].rearrange("p (h d) -> p h d", h=NH))
                        nc.vector.tensor_copy(
                            v_sb[tt][:, :, HD:HD + 1],
                            ones_st[:, 0:NH].rearrange("p (h o) -> p h o", o=1))

                # ================= phase 2: attention =================
                ph2 = ph12.enter_context(ExitStack())
                sps = ph2.enter_context(
                    tc.tile_pool(name="sps", bufs=2, space="PSUM"))
                yps = ph2.enter_context(
                    tc.tile_pool(name="yps", bufs=2, space="PSUM"))
                ppool = ph2.enter_context(tc.tile_pool(name="ppool", bufs=3))
                npool = ph2.enter_context(tc.tile_pool(name="npool", bufs=2))

                for h in range(NH):
                    ct_q, ct_k = h // 2, 4 + h // 2
                    p0 = 64 * (h % 2)
                    qT_h = qkT[ct_q][p0:p0 + 64, :]
                    kT_h = qkT[ct_k][p0:p0 + 64, :]
                    for qh in range(2):
                        qbase = 1024 * qh
                        y_ps = yps.tile([65, 1024], F32, tag="y")
                        n_k = 8 * qh + 8
                        for i in range(n_k):
                            q0 = max(qbase, 128 * i)
                            w = qbase + 1024 - q0
                            s_ps = sps.tile([128, w], F32, tag="s")
                            for off, n in _pieces(w):
                                nc.tensor.matmul(
                                    s_ps[:, off:off + n],
                                    kT_h[:, 128 * i:128 * (i + 1)],
                                    qT_h[:, q0 + off:q0 + off + n],
                                    start=True, stop=True)
                            p_sb = ppool.tile([128, 1024], F32R, tag="p")
                            nc.scalar.activation(
                                p_sb[:, 0:w], s_ps[:], AF.Exp, scale=SCALE)
                            if 128 * i >= qbase:  # diagonal block: causal mask
                                nc.vector.tensor_mul(
                                    p_sb[:, 0:128], p_sb[:, 0:128], mask_r[:])
                            # AV pieces: align to y_ps's 512-wide PSUM banks
                            yoff = q0 - qbase
                            aoff = 0
                            while aoff < w:
                                n = min(512 - (yoff + aoff) % 512, w - aoff)
                                bank = (yoff + aoff) // 512
                                nc.tensor.matmul(
                                    y_ps[:, yoff + aoff:yoff + aoff + n],
                                    v_sb[i][:, h, :],
                                    p_sb[:, aoff:aoff + n],
                                    start=(i == 0),
                                    stop=(i == 8 * qh + 4 * bank + 3))
                                aoff += n
                        # normalize: yT[h//2][64*(h%2):, qbase:+1024] = y/l
                        # l lives on lane 64; broadcast to lanes 0-63 via a
                        # ones-stationary PE matmul (1-partition contraction),
                        # then reciprocal on all 64 lanes at once.
                        l_r = npool.tile([65, 1024], F32R, tag="lr")
                        nc.vector.tensor_copy(l_r[64:65, :], y_ps[64:65, :])
                        rb_ps = sps.tile([64, 1024], F32, tag="s")
                        for off in (0, 512):
                            nc.tensor.matmul(
                                rb_ps[:, off:off + 512],
                                ones64[64:65, :],
                                l_r[64:65, off:off + 512],
                                start=True, stop=True)
                        recip_b = npool.tile([64, 1024], F32, tag="recipb")
                        nc.vector.reciprocal(recip_b[:], rb_ps[:])
                        if h % 2 == 0:
                            nc.vector.tensor_mul(
                                yT[h // 2][0:64, qbase:qbase + 1024],
                                y_ps[0:64, :], recip_b[:])
                        else:
                            ytmp = npool.tile([64, 1024], F32R, tag="ytmp")
                            nc.vector.tensor_mul(
                                ytmp[:], y_ps[0:64, :], recip_b[:])
                            nc.sync.dma_start(
                                yT[h // 2][64:128, qbase:qbase + 1024],
                                ytmp[:])

                ph2.close()
                ph12.close()  # free qkT/v_sb before proj

                # ================= phase 3: proj =================
                ph3 = ph23.enter_context(ExitStack())
                p3 = ph3.enter_context(tc.tile_pool(name="p3", bufs=1))
                ops = ph3.enter_context(
                    tc.tile_pool(name="ops", bufs=6, space="PSUM"))
                stpool = ph3.enter_context(tc.tile_pool(name="stpool", bufs=2))

                wp_bf = p3.tile([128, 4, D], BF16, tag="wpb")
                wp_r = p3.tile([128, 4, D], F32R, tag="wpr")
                nc.sync.dma_start(wp_bf[:], wp_ext)
                nc.vector.tensor_copy(wp_r[:], wp_bf[:])

                for grp in range(4):
                    stage = stpool.tile([128, 4, D], F32, tag="stg")
                    for j in range(4):
                        tt = grp * 4 + j
                        o_ps = ops.tile([128, 512], F32, tag="o")
                        o_ps2 = ops.tile([128, 512], F32, tag="o")
                        for oc, ps in enumerate([o_ps, o_ps2]):
                            for dc in range(4):
                                nc.tensor.matmul(
                                    ps[:],
                                    yT[dc][:, tt * 128:(tt + 1) * 128],
                                    wp_r[:, dc, oc * 512:(oc + 1) * 512],
                                    start=(dc == 0), stop=(dc == 3))
                        nc.any.tensor_copy(stage[:, j, 0:512], o_ps[:])
                        nc.any.tensor_copy(stage[:, j, 512:1024], o_ps2[:])
                    nc.sync.dma_start(
                        out_ext[:, grp * 4:(grp + 1) * 4, :], stage[:])

        if repeat == 1:
            body()
        else:
            with tc.For_i(0, repeat, 1) as iv:
                body(iv)

    nc.compile()
    return nc


def make_in_maps(x, W_attn, W_proj):
    trimask = np.triu(np.ones((128, 128), dtype=np.float32)).astype(BF16_NP)
    # xT packed per batch: [128, 8, 2048], xTp[p, dc, t] = x[b, t, dc*128+p]
    xTs = []
    for b in range(B):
        xT = np.ascontiguousarray(
            x[b].T.reshape(DC, 128, T).transpose(1, 0, 2)).astype(BF16_NP)
        xTs.append(xT)
    in_maps = []
    for c in range(8):
        b, g = c // 2, c % 2
        # q cols then k cols for this head group, as 8 col-tiles of 128
        qk_cols = np.concatenate(
            [W_attn[:, 512 * g:512 * g + 512],
             W_attn[:, 1024 + 512 * g:1024 + 512 * g + 512]], axis=1)
        # wqk[p, ct, dc, n] = qk_cols[dc*128+p, ct*128+n]
        wqk = np.ascontiguousarray(
            qk_cols.reshape(DC, 128, 8, 128).transpose(1, 2, 0, 3)
        ).astype(BF16_NP)
        v_cols = W_attn[:, 2048 + 512 * g:2048 + 512 * g + 512]
        wav = np.ascontiguousarray(
            v_cols.reshape(DC, 128, 512).transpose(1, 0, 2)).astype(BF16_NP)
        wp = np.ascontiguousarray(
            W_proj[512 * g:512 * (g + 1), :].reshape(4, 128, D).transpose(1, 0, 2)
        ).astype(BF16_NP)
        in_maps.append({
            "xT": xTs[b],
            "wqk": wqk,
            "wav": wav,
            "wp": wp,
            "trimask": trimask,
        })
    return in_maps


_NC_CACHE = {}


def kernel(x, W_attn, W_proj):
    x = np.asarray(x, dtype=np.float32)
    W_attn = np.asarray(W_attn, dtype=np.float32)
    W_proj = np.asarray(W_proj, dtype=np.float32)
    if "nc" not in _NC_CACHE:
        _NC_CACHE["nc"] = build()
    nc = _NC_CACHE["nc"]
    in_maps = make_in_maps(x, W_attn, W_proj)
    res = run_bass_kernel_spmd(nc, in_maps, list(range(8)))
    out = np.empty((B, T, D), dtype=np.float32)
    for b in range(B):
        # out_shard[p, tt, n] = partial_out[tt*128+p, n]; pair-sum = unshard
        acc = res.results[2 * b]["out_shard"] + res.results[2 * b + 1]["out_shard"]
        out[b] = acc.transpose(1, 0, 2).reshape(T, D)
    return out


# revision 13
# speedup vs baseline: 1.8473x; 1.2148x over previous
"""Causal self-attention on 8 TRN2 NeuronCores — no-collective, f32r attn.

Sharding: batch x head-group mesh (4 x 2). Core c handles batch b = c//2
and head group g = c%2 (8 of 16 heads). Each core computes its partial
projection out_partial = y(8 heads) @ W_proj[rows(g)] over the FULL
sequence; the host sums the two partials of each pair while unsharding.
No on-device collective -> cores are fully independent.

Dtypes: host inputs are bf16 (halves DMA). QKV matmuls run bf16
(Ldweights per matmul, hidden under phase-1 copies). Everything after
the QKV PSUM is float32r: q/k/v land in F32R tiles (the PSUM->SBUF copy
does the f32r encode at no extra cost), exp writes F32R, so every
attention and projection matmul is a self-loading f32r matmul — no
standalone Ldweights and full-rate moving streams.

Host-side packing (part of sharding):
  xT   [128, 8, 2048] bf16 : x[b].T packed partition-major
  wqk  [128, 8, 8, 128] bf16: q/k weight col-tiles, stationary-ready
  wav  [128, 8, 512] bf16  : v weights, contraction packed
  wp   [128, 4, 1024] bf16 : W_proj rows(g) packed
  out  [128, 16, 1024] f32 : partial projection, partition-major rows

Per head: S^T = k^T.T @ q^T, P = exp(S/8) causal, y = P@v with a ones
column giving the softmax denominator l for free; l broadcast across
partitions via a ones-stationary PE matmul, reciprocal on 64 lanes.
"""

from contextlib import ExitStack

import numpy as np
import ml_dtypes

import concourse.bass as bass
import concourse.tile as tile
from concourse import bacc, mybir
from concourse.bass_utils import run_bass_kernel_spmd

F32 = mybir.dt.float32
F32R = mybir.dt.float32r
BF16 = mybir.dt.bfloat16
AF = mybir.ActivationFunctionType
BF16_NP = np.dtype(ml_dtypes.bfloat16)

D = 1024          # model dim
T = 2048          # sequence length
B = 4             # batch
HD = 64           # head dim
NH = 8            # heads per core
DC = D // 128     # 8 contraction chunks
TT = T // 128     # 16 t-tiles
SCALE = 1.0 / 8.0  # 1/sqrt(HD)


def _pieces(width):
    """Split width into matmul pieces <=512, PSUM-bank aligned from offset 0."""
    out = []
    off = 0
    while off < width:
        n = min(512, width - off)
        out.append((off, n))
        off += n
    return out


def build(repeat=1, collective=False):
    nc = bacc.Bacc("TRN2", target_bir_lowering=False, debug=False, num_devices=1)

    xT_ext = nc.dram_tensor("xT", [128, DC, T], BF16, kind="ExternalInput").ap()
    wqk_ext = nc.dram_tensor("wqk", [128, 8, DC, 128], BF16, kind="ExternalInput").ap()
    wav_ext = nc.dram_tensor("wav", [128, DC, 512], BF16, kind="ExternalInput").ap()
    wp_ext = nc.dram_tensor("wp", [128, 4, D], BF16, kind="ExternalInput").ap()
    mask_ext = nc.dram_tensor("trimask", [128, 128], BF16, kind="ExternalInput").ap()
    out_ext = nc.dram_tensor("out_shard", [128, TT, D], F32, kind="ExternalOutput").ap()

    with tile.TileContext(nc) as tc, ExitStack() as top:
        # ---- persistent tiles ----
        pers = top.enter_context(tc.tile_pool(name="pers", bufs=1))

        mask_bf = pers.tile([128, 128], BF16, tag="maskbf")
        mask_r = pers.tile([128, 128], F32R, tag="maskr")
        ones_st = pers.tile([128, 64], F32, tag="ones_st")
        ones64 = pers.tile([128, 64], F32R, tag="ones64")

        nc.sync.dma_start(mask_bf[:], mask_ext)
        nc.vector.tensor_copy(mask_r[:], mask_bf[:])
        nc.vector.memset(ones_st[:], 1.0)
        nc.vector.tensor_copy(ones64[:], ones_st[:])

        def body(iv=None):
            with ExitStack() as ph23:
                p23 = ph23.enter_context(tc.tile_pool(name="p23", bufs=1))
                yT = [p23.tile([128, T], F32R, tag=f"yT{i}", name=f"yT{i}")
                      for i in range(4)]

                ph12 = ph23.enter_context(ExitStack())
                p12 = ph12.enter_context(tc.tile_pool(name="p12", bufs=1))
                # qkT in bf16 so xT/wqk can stay resident through the first
                # attention half (SBUF budget) for the interleaved qk-B units
                qkT = [p12.tile([128, T], BF16, tag=f"qkT{i}", name=f"qkT{i}")
                       for i in range(8)]
                # v_sb[tt]: [128 k-parts, 8 heads, 64 v + 1 ones] f32r
                v_sb = [p12.tile([128, NH, HD + 1], F32R, tag=f"v{i}", name=f"v{i}")
                        for i in range(TT)]

                # px: input tiles, live until attention of heads 0-3 is done
                px = ph12.enter_context(ExitStack())
                pxp = px.enter_context(tc.tile_pool(name="px", bufs=1))
                xT = pxp.tile([128, DC, T], BF16, tag="xT", name="xT")
                wqk = pxp.tile([128, 8, DC, 128], BF16, tag="wqk", name="wqk")
                wav = pxp.tile([128, DC, 512], BF16, tag="wav", name="wav")
                nc.sync.dma_start(wqk[:], wqk_ext)
                nc.sync.dma_start(xT[:], xT_ext)
                nc.sync.dma_start(wav[:], wav_ext)

                # ========= phase 1a: q/k for heads 0-3, all of v =========
                with ExitStack() as ph1:
                    qkvps = ph1.enter_context(
                        tc.tile_pool(name="qkvps", bufs=3, space="PSUM"))

                    for ct in [0, 4, 1, 5]:
                        for tch in range(4):
                            ps = qkvps.tile([128, 512], F32, tag="qkvps")
                            for dc in range(DC):
                                nc.tensor.matmul(
                                    ps[:],
                                    wqk[:, ct, dc, :],
                                    xT[:, dc, tch * 512:(tch + 1) * 512],
                                    start=(dc == 0), stop=(dc == DC - 1))
                            nc.vector.tensor_copy(
                                qkT[ct][:, tch * 512:(tch + 1) * 512], ps[:])

                    # --- v natural: v[t-tile] = x[t-tile] @ wa_v ---
                    for tt in range(TT):
                        ps = qkvps.tile([128, 512], F32, tag="qkvps")
                        for dc in range(DC):
                            nc.tensor.matmul(
                                ps[:],
                                xT[:, dc, tt * 128:(tt + 1) * 128],
                                wav[:, dc, :],
                                start=(dc == 0), stop=(dc == DC - 1))
                        nc.vector.tensor_copy(
                            v_sb[tt][:, :, 0:HD],
                            ps[:].rearrange("p (h d) -> p h d", h=NH))
                        nc.vector.tensor_copy(
                            v_sb[tt][:, :, HD:HD + 1],
                            ones_st[:, 0:NH].rearrange("p (h o) -> p h o", o=1))

                # ================= phase 2: attention =================
                # q/k for heads 4-7 are emitted as 16 units interleaved
                # between attention blocks of heads 0-3: the exp (ACT)
                # stream is the wall there, so these matmuls fill idle PE.
                ph2 = ph12.enter_context(ExitStack())
                sps = ph2.enter_context(
                    tc.tile_pool(name="sps", bufs=2, space="PSUM"))
                yps = ph2.enter_context(
                    tc.tile_pool(name="yps", bufs=2, space="PSUM"))
                ppool = ph2.enter_context(tc.tile_pool(name="ppool", bufs=3))
                npool = ph2.enter_context(tc.tile_pool(name="npool", bufs=2))

                bunits = [(ct, tch) for ct in [2, 6, 3, 7] for tch in range(4)]
                bstate = {"i": 0}

                def qkb_unit():
                    ct, tch = bunits[bstate["i"]]
                    bstate["i"] += 1
                    ps = sps.tile([128, 512], F32, tag="s")
                    for dc in range(DC):
                        nc.tensor.matmul(
                            ps[:],
                            wqk[:, ct, dc, :],
                            xT[:, dc, tch * 512:(tch + 1) * 512],
                            start=(dc == 0), stop=(dc == DC - 1))
                    nc.vector.tensor_copy(
                        qkT[ct][:, tch * 512:(tch + 1) * 512], ps[:])

                for h in range(NH):
                    ct_q, ct_k = h // 2, 4 + h // 2
                    p0 = 64 * (h % 2)
                    qT_h = qkT[ct_q][p0:p0 + 64, :]
                    kT_h = qkT[ct_k][p0:p0 + 64, :]
                    for qh in range(2):
                        qbase = 1024 * qh
                        y_ps = yps.tile([65, 1024], F32, tag="y")
                        n_k = 8 * qh + 8
                        for i in range(n_k):
                            q0 = max(qbase, 128 * i)
                            w = qbase + 1024 - q0
                            s_ps = sps.tile([128, w], F32, tag="s")
                            for off, n in _pieces(w):
                                nc.tensor.matmul(
                                    s_ps[:, off:off + n],
                                    kT_h[:, 128 * i:128 * (i + 1)],
                                    qT_h[:, q0 + off:q0 + off + n],
                                    start=True, stop=True)
                            p_sb = ppool.tile([128, 1024], F32R, tag="p")
                            nc.scalar.activation(
                                p_sb[:, 0:w], s_ps[:], AF.Exp, scale=SCALE)
                            if 128 * i >= qbase:  # diagonal block: causal mask
                                nc.vector.tensor_mul(
                                    p_sb[:, 0:128], p_sb[:, 0:128], mask_r[:])
                            # AV pieces: align to y_ps's 512-wide PSUM banks
                            yoff = q0 - qbase
                            aoff = 0
                            while aoff < w:
                                n = min(512 - (yoff + aoff) % 512, w - aoff)
                                bank = (yoff + aoff) // 512
                                nc.tensor.matmul(
                                    y_ps[:, yoff + aoff:yoff + aoff + n],
                                    v_sb[i][:, h, :],
                                    p_sb[:, aoff:aoff + n],
                                    start=(i == 0),
                                    stop=(i == 8 * qh + 4 * bank + 3))
                                aoff += n
                        # normalize: yT[h//2][64*(h%2):, qbase:+1024] = y/l
                        # l lives on lane 64; broadcast to lanes 0-63 via a
                        # ones-stationary PE matmul (1-partition contraction),
                        # then reciprocal on all 64 lanes at once.
                        l_r = npool.tile([65, 1024], F32R, tag="lr")
                        nc.vector.tensor_copy(l_r[64:65, :], y_ps[64:65, :])
                        rb_ps = sps.tile([64, 1024], F32, tag="s")
                        for off in (0, 512):
                            nc.tensor.matmul(
                                rb_ps[:, off:off + 512],
                                ones64[64:65, :],
                                l_r[64:65, off:off + 512],
                                start=True, stop=True)
                        recip_b = npool.tile([64, 1024], F32, tag="recipb")
                        nc.vector.reciprocal(recip_b[:], rb_ps[:])
                        if h % 2 == 0:
                            nc.vector.tensor_mul(
                                yT[h // 2][0:64, qbase:qbase + 1024],
                                y_ps[0:64, :], recip_b[:])
                        else:
                            ytmp = npool.tile([64, 1024], F32R, tag="ytmp")
                            nc.vector.tensor_mul(
                                ytmp[:], y_ps[0:64, :], recip_b[:])
                            nc.sync.dma_start(
                                yT[h // 2][64:128, qbase:qbase + 1024],
                                ytmp[:])
                        for _ in range(2):  # 16 units over 8 blocks of h 0-3
                            if bstate["i"] < len(bunits):
                                qkb_unit()
                    if h == 3:
                        assert bstate["i"] == len(bunits)
                        # px (xT/wqk/wav) is released by ph12.close() below;
                        # closing it here would violate pool LIFO order.

                ph2.close()
                ph12.close()  # free qkT/v_sb before proj

                # ================= phase 3: proj =================
                ph3 = ph23.enter_context(ExitStack())
                p3 = ph3.enter_context(tc.tile_pool(name="p3", bufs=1))
                ops = ph3.enter_context(
                    tc.tile_pool(name="ops", bufs=6, space="PSUM"))
                stpool = ph3.enter_context(tc.tile_pool(name="stpool", bufs=2))

                wp_bf = p3.tile([128, 4, D], BF16, tag="wpb")
                wp_r = p3.tile([128, 4, D], F32R, tag="wpr")
                nc.sync.dma_start(wp_bf[:], wp_ext)
                nc.vector.tensor_copy(wp_r[:], wp_bf[:])

                for grp in range(4):
                    stage = stpool.tile([128, 4, D], F32, tag="stg")
                    for j in range(4):
                        tt = grp * 4 + j
                        o_ps = ops.tile([128, 512], F32, tag="o")
                        o_ps2 = ops.tile([128, 512], F32, tag="o")
                        for oc, ps in enumerate([o_ps, o_ps2]):
                            for dc in range(4):
                                nc.tensor.matmul(
                                    ps[:],
                                    yT[dc][:, tt * 128:(tt + 1) * 128],
                                    wp_r[:, dc, oc * 512:(oc + 1) * 512],
                                    start=(dc == 0), stop=(dc == 3))
                        nc.any.tensor_copy(stage[:, j, 0:512], o_ps[:])
                        nc.any.tensor_copy(stage[:, j, 512:1024], o_ps2[:])
                    nc.sync.dma_start(
                        out_ext[:, grp * 4:(grp + 1) * 4, :], stage[:])

        if repeat == 1:
            body()
        else:
            with tc.For_i(0, repeat, 1) as iv:
                body(iv)

    nc.compile()
    return nc


def make_in_maps(x, W_attn, W_proj):
    trimask = np.triu(np.ones((128, 128), dtype=np.float32)).astype(BF16_NP)
    # xT packed per batch: [128, 8, 2048], xTp[p, dc, t] = x[b, t, dc*128+p]
    xTs = []
    for b in range(B):
        xT = np.ascontiguousarray(
            x[b].T.reshape(DC, 128, T).transpose(1, 0, 2)).astype(BF16_NP)
        xTs.append(xT)
    in_maps = []
    for c in range(8):
        b, g = c // 2, c % 2
        # q cols then k cols for this head group, as 8 col-tiles of 128
        qk_cols = np.concatenate(
            [W_attn[:, 512 * g:512 * g + 512],
             W_attn[:, 1024 + 512 * g:1024 + 512 * g + 512]], axis=1)
        # wqk[p, ct, dc, n] = qk_cols[dc*128+p, ct*128+n]
        wqk = np.ascontiguousarray(
            qk_cols.reshape(DC, 128, 8, 128).transpose(1, 2, 0, 3)
        ).astype(BF16_NP)
        v_cols = W_attn[:, 2048 + 512 * g:2048 + 512 * g + 512]
        wav = np.ascontiguousarray(
            v_cols.reshape(DC, 128, 512).transpose(1, 0, 2)).astype(BF16_NP)
        wp = np.ascontiguousarray(
            W_proj[512 * g:512 * (g + 1), :].reshape(4, 128, D).transpose(1, 0, 2)
        ).astype(BF16_NP)
        in_maps.append({
            "xT": xTs[b],
            "wqk": wqk,
            "wav": wav,
            "wp": wp,
            "trimask": trimask,
        })
    return in_maps


_NC_CACHE = {}


def kernel(x, W_attn, W_proj):
    x = np.asarray(x, dtype=np.float32)
    W_attn = np.asarray(W_attn, dtype=np.float32)
    W_proj = np.asarray(W_proj, dtype=np.float32)
    if "nc" not in _NC_CACHE:
        _NC_CACHE["nc"] = build()
    nc = _NC_CACHE["nc"]
    in_maps = make_in_maps(x, W_attn, W_proj)
    res = run_bass_kernel_spmd(nc, in_maps, list(range(8)))
    out = np.empty((B, T, D), dtype=np.float32)
    for b in range(B):
        # out_shard[p, tt, n] = partial_out[tt*128+p, n]; pair-sum = unshard
        acc = res.results[2 * b]["out_shard"] + res.results[2 * b + 1]["out_shard"]
        out[b] = acc.transpose(1, 0, 2).reshape(T, D)
    return out
